# revision 40
# baseline (speedup 1.0000x reference)
"""MetaPathGNN Trainium2 kernel: 8-core SPMD, node-sharded.

Host (untimed): edge filtering/sorting/partitioning, weight folding, layout prep.
Device: feature-major MLP, AllGather of projected messages, dma_gather of source
rows, PE one-hot matmul segment-sum (PSUM accumulation per 128-dst window),
classifier + log_softmax.
"""

import hashlib
import sys

import numpy as np

sys.path.insert(0, "/opt/trn_rl_repo")

import concourse.bass as bass
import concourse.bacc as bacc
import concourse.mybir as mybir
from concourse.bass_utils import run_bass_kernel_spmd
from concourse.tile import TileContext

N = 50000
P = 8
NPC = 6250          # nodes per core
NPP = 6272          # padded: 49 * 128
NT = NPP // 128     # 49 node tiles / dst windows per core
D = 128
H2 = 256
NCLS = 40
REL0, REL1 = 2, 3
HALF = 4 * NPP      # 25088: int16 gather index range split
CHUNK = 1024        # gather chunk (descriptor ring tops out < 2048)

F32 = mybir.dt.float32
F32R = mybir.dt.float32r
BF16 = mybir.dt.bfloat16
I16 = mybir.dt.int16

import os
REPEAT = int(os.environ.get("KREPEAT", "1"))
SKIP_AG = os.environ.get("SKIP_AG") == "1"
SKIP_GATHER = os.environ.get("SKIP_GATHER") == "1"
SKIP_GRAPH = os.environ.get("SKIP_GRAPH") == "1"
HWLOOP = os.environ.get("HWLOOP") == "1"
SMALL_AG = os.environ.get("SMALL_AG") == "1"
_CACHE = {}
LAST_EXEC_NS = None
LAST_RESULTS = None
TRACE = False
TRACE_KW = {}


def _wrap_idx(a):
    """[L] int16 -> [128, L/16] in (s p) wrapped layout, replicated for 8 q7 cores."""
    sb = a.reshape(-1, 16).T.copy()
    return np.tile(sb, (8, 1))


def _prep_edges(edge_index, edge_type):
    """Per (layer, half): uniform-cap window-sorted edge streams.

    Stream = concat over dst-window w of that window's edges, padded per window
    to cap_w (max count over cores) with (src=0, dstloc=-1) null edges; total
    padded to a CHUNK multiple (tail assigned to the last window).
    Returns dict[(layer, half)] -> (L, bounds, per_core list of (srel, dstloc)).
    bounds[w] = start position of window w in the stream (static, shared).
    """
    ei = np.asarray(edge_index)
    et = np.asarray(edge_type)
    dst_all = ei[0].astype(np.int64)
    src_all = ei[1].astype(np.int64)
    out = {}
    for layer, rel in ((0, REL0), (1, REL1)):
        sel = et == rel
        dst = dst_all[sel]
        src = src_all[sel]
        srow = (src // NPC) * NPP + (src % NPC)
        groups = [[[None] * NT for _ in range(P)] for _ in range(2)]
        for c in range(P):
            m = (dst >= c * NPC) & (dst < (c + 1) * NPC)
            d_loc = (dst[m] - c * NPC).astype(np.int64)
            s_row = srow[m]
            for half in (0, 1):
                hm = (s_row < HALF) if half == 0 else (s_row >= HALF)
                sr = s_row[hm] - half * HALF
                dl = d_loc[hm]
                w = dl // 128
                order = np.argsort(w, kind="stable")
                sr, dl, w = sr[order], dl[order], w[order]
                idx = np.searchsorted(w, np.arange(NT + 1))
                for wi in range(NT):
                    groups[half][c][wi] = (sr[idx[wi]:idx[wi + 1]],
                                           dl[idx[wi]:idx[wi + 1]])
        for half in (0, 1):
            caps = [max(len(groups[half][c][w][0]) for c in range(P))
                    for w in range(NT)]
            L = sum(caps)
            Lpad = ((L + CHUNK - 1) // CHUNK) * CHUNK
            caps[-1] += Lpad - L
            bounds = np.concatenate([[0], np.cumsum(caps)])
            lists = []
            for c in range(P):
                srel = np.zeros(Lpad, np.int64)
                dloc = np.full(Lpad, -1, np.int64)
                for w in range(NT):
                    sr, dl = groups[half][c][w]
                    b = bounds[w]
                    srel[b:b + len(sr)] = sr
                    dloc[b:b + len(dl)] = dl
                lists.append((srel, dloc))
            out[(layer, half)] = (Lpad, bounds, lists)
    return out


def _prep_inputs(inputs):
    f = lambda k: np.asarray(inputs[k], dtype=np.float32)
    x = f("x")
    edges = _prep_edges(inputs["edge_index"], inputs["edge_type"])

    w1, b1 = f("mlp_w1"), f("mlp_b1")
    w2, b2 = f("mlp_w2"), f("mlp_b2")
    w3, b3 = f("mlp_w3"), f("mlp_b3")
    w01_0 = f("w0_0") + f("w1_0")
    ball0 = f("b0_0") + f("b1_0") + f("bl_0")
    w01_1 = f("w0_1") + f("w1_1")
    ball1 = f("b0_1") + f("b1_1") + f("bl_1")
    wl0, wl1 = f("wl_0"), f("wl_1")
    fc1s = f("fc1_w")[:D] + f("fc1_w")[D:]
    fc1b = f("fc1_b")
    fc2w, fc2b = f("fc2_w"), f("fc2_b")
    wcat0 = np.concatenate([wl0, w01_0], axis=1)   # [256, 256] -> [m0 | d0]
    wcat1 = np.concatenate([wl1, w01_1], axis=1)   # [128, 256] -> [m1 | d1]

    import ml_dtypes
    bf = lambda a: np.ascontiguousarray(a).astype(ml_dtypes.bfloat16)
    iota = np.tile(np.arange(128, dtype=np.float32), (128, 1))
    shared = {
        "w1": w1, "w2": w2, "w3": w3,
        "b1": b1.reshape(D, 1), "b2": b2.reshape(D, 1),
        "b3a": b3[:D].reshape(D, 1), "b3b": b3[D:].reshape(D, 1),
        "wl0a": bf(wl0[:D]),
        "wl0b": bf(wl0[D:]),
        "w01a": np.ascontiguousarray(w01_0[:D]),
        "w01b": np.ascontiguousarray(w01_0[D:]),
        "wl1": bf(wl1), "w011": bf(w01_1),
        "ball0": ball0.reshape(D, 1), "ball1": ball1.reshape(D, 1),
        "fc1s": bf(fc1s), "fc1b": fc1b.reshape(D, 1),
        "fc2w": fc2w, "fc2b": fc2b.reshape(NCLS, 1),
        "ones40": np.ones((NCLS, 1), np.float32), "ones1x40": np.ones((1, NCLS), np.float32),
        "iota128": iota[:, None, :].copy(),
    }
    meta = {k: (v[0], v[1]) for k, v in edges.items()}
    # enumerate one-hot ops (w, half, tile) exactly as _build does
    sops = {}
    for layer in (0, 1):
        ops = []
        for w in range(NT):
            for half in (0, 1):
                Lpad, bounds, lists = edges[(layer, half)]
                t0 = bounds[w] // 128
                t1 = (bounds[w + 1] - 1) // 128
                for t in range(t0, t1 + 1):
                    ops.append((w, half, t))
        sops[layer] = ops

    in_maps = []
    for c in range(P):
        m = dict(shared)
        xt = np.zeros((D, NPP), np.float32)
        xt[:, :NPC] = x[c * NPC:(c + 1) * NPC].T
        m["xt"] = xt
        for (layer, half), (Lpad, bounds, lists) in edges.items():
            srel, dloc = lists[c]
            m[f"gs{layer}{half}"] = _wrap_idx(srel.astype(np.int16))
        for layer in (0, 1):
            ops = sops[layer]
            drel = np.full((128, len(ops)), -1.0, np.float32)
            for i, (w, half, t) in enumerate(ops):
                dloc = edges[(layer, half)][2][c][1][t * 128:(t + 1) * 128]
                rel = dloc - 128 * w
                valid = (rel >= 0) & (rel < 128)
                drel[valid, i] = rel[valid]
            m[f"dr{layer}"] = drel
        in_maps.append(m)
    return in_maps, meta


def _build(meta):
    nc = bacc.Bacc(None, target_bir_lowering=False, num_swdge_queues=4)

    def din(name, shape, dtype=F32):
        return nc.dram_tensor(name, list(shape), dtype, kind="ExternalInput")

    BF16_W = {"wl0a", "wl0b", "wl1", "w011", "fc1s"}
    F32R_W = {"w1", "w2", "w3", "w01a", "w01b", "fc2w", "ones40", "ones1x40"}
    xt_d = din("xt", (D, NPP), F32R)
    wd = {}
    for name, shape in [
        ("w1", (D, D)), ("w2", (D, D)), ("w3", (D, H2)),
        ("b1", (D, 1)), ("b2", (D, 1)), ("b3a", (D, 1)), ("b3b", (D, 1)),
        ("wl0a", (D, D)), ("wl0b", (D, D)), ("w01a", (D, D)), ("w01b", (D, D)),
        ("wl1", (D, D)), ("w011", (D, D)),
        ("ball0", (D, 1)), ("ball1", (D, 1)),
        ("fc1s", (D, D)), ("fc1b", (D, 1)),
        ("fc2w", (D, NCLS)), ("fc2b", (NCLS, 1)),
        ("ones40", (NCLS, 1)), ("ones1x40", (1, NCLS)),
        ("iota128", (D, 1, D)),
    ]:
        dt = BF16 if name in BF16_W else (F32R if name in F32R_W else F32)
        wd[name] = din(name, shape, dt)
    idx_d = {}
    for (layer, half), (Lpad, bounds) in meta.items():
        idx_d[(layer, half, "s")] = din(f"gs{layer}{half}", (128, Lpad // 16), I16)
    nops = {}
    for layer in (0, 1):
        ops = []
        for w in range(NT):
            for half in (0, 1):
                Lpad, bounds = meta[(layer, half)]
                t0 = bounds[w] // 128
                t1 = (bounds[w + 1] - 1) // 128
                for t in range(t0, t1 + 1):
                    ops.append((w, half, t))
        nops[layer] = ops
        idx_d[(layer, "dr")] = din(f"dr{layer}", (128, len(ops)), F32)

    m_own = [nc.dram_tensor(f"m{i}_own", [NPP, D], BF16) for i in range(2)]
    m_full = [
        nc.dram_tensor(f"m{i}_full", [P * NPP, D], BF16, addr_space="Shared")
        for i in range(2)
    ]
    y_d = nc.dram_tensor("y", [NCLS, NPP], F32, kind="ExternalOutput")

    AF = mybir.ActivationFunctionType
    ALU = mybir.AluOpType
    NCH = 13

    def chunks512():
        for i in range(NCH):
            lo = i * 512
            yield lo, min(512, NPP - lo)

    with TileContext(nc) as tc:
        with tc.tile_pool(name="const", bufs=1) as cpool:
            W = {}
            for name, t in wd.items():
                dt = BF16 if name in BF16_W else (F32R if name in F32R_W else F32)
                W[name] = cpool.tile(list(t.shape), dt, tag=name, name=f"W_{name}")
                nc.sync.dma_start(out=W[name][:], in_=t[:])

            def body(rep):
              with tc.tile_pool(name=f"persist{rep}", bufs=1) as pp:
                dterm = pp.tile([128, NPP], F32, name="dterm")     # feature-major
                out_fm = pp.tile([128, NPP], BF16, name="out_fm")  # feature-major

                # ---------------- Phase 1: MLP ----------------
                with (
                    tc.tile_pool(name=f"mlp{rep}", bufs=1) as mp,
                    tc.tile_pool(name=f"mlpc{rep}", bufs=4) as mpc,
                ):
                    xt = mp.tile([D, NPP], F32R, name="xt_s")
                    nc.sync.dma_start(out=xt[:], in_=xt_d[:])
                    h3 = [mp.tile([D, NPP], F32R, name=f"h3_{j}") for j in range(2)]
                    h3b = [mp.tile([D, NPP], BF16, name=f"h3b_{j}") for j in range(2)]
                    with tc.tile_pool(name=f"psA{rep}", bufs=2, space="PSUM") as psA:
                        for lo, w in chunks512():
                            ps1 = psA.tile([D, 512], F32, tag="ps1", name="ps1")
                            nc.tensor.matmul(ps1[:, :w], W["w1"][:], xt[:, lo:lo + w])
                            h1 = mpc.tile([D, 512], F32R, tag="h1", name="h1")
                            nc.scalar.activation(h1[:, :w], ps1[:, :w], AF.Relu,
                                                 bias=W["b1"][:])
                            ps2 = psA.tile([D, 512], F32, tag="ps2", name="ps2")
                            nc.tensor.matmul(ps2[:, :w], W["w2"][:], h1[:, :w])
                            h2t = mpc.tile([D, 512], F32R, tag="h2", name="h2")
                            nc.scalar.activation(h2t[:, :w], ps2[:, :w], AF.Relu,
                                                 bias=W["b2"][:])
                            for j in range(2):
                                ps3 = psA.tile([D, 512], F32, tag="ps3", name="ps3")
                                nc.tensor.matmul(
                                    ps3[:, :w], W["w3"][:, j * D:(j + 1) * D],
                                    h2t[:, :w]
                                )
                                nc.scalar.activation(
                                    h3[j][:, lo:lo + w], ps3[:, :w], AF.Identity,
                                    bias=W["b3a"][:] if j == 0 else W["b3b"][:],
                                )
                                nc.vector.tensor_copy(h3b[j][:, lo:lo + w],
                                                      h3[j][:, lo:lo + w])
                    # m0 node-major (for gather rows) + d0 feature-major into dterm
                    with (
                        tc.tile_pool(name=f"md0{rep}", bufs=6) as md0p,
                        tc.tile_pool(name=f"psB{rep}", bufs=4, space="PSUM") as psB,
                    ):
                        for lo, w in chunks512():
                            psd = psB.tile([D, 512], F32, tag="d0ps", name="d0ps")
                            nc.tensor.matmul(psd[:, :w], W["w01a"][:], h3[0][:, lo:lo + w],
                                             start=True, stop=False)
                            nc.tensor.matmul(psd[:, :w], W["w01b"][:], h3[1][:, lo:lo + w],
                                             start=False, stop=True)
                            nc.scalar.activation(dterm[:, lo:lo + w], psd[:, :w],
                                                 AF.Identity, bias=W["ball0"][:])
                        m_own0_t = m_own[0].reshape([NT, 128, D])
                        for t0g in range(0, NT, 4):
                            tg = list(range(t0g, min(t0g + 4, NT)))
                            g = len(tg)
                            ps = psB.tile([128, 4, 128], F32, tag="m0ps", name="m0ps")
                            for j, t in enumerate(tg):
                                lo = t * 128
                                nc.tensor.matmul(ps[:, j, :],
                                                 h3b[0][:, lo:lo + 128], W["wl0a"][:],
                                                 start=True, stop=False,
                                                 skip_group_check=True)
                                nc.tensor.matmul(ps[:, j, :],
                                                 h3b[1][:, lo:lo + 128], W["wl0b"][:],
                                                 start=False, stop=True,
                                                 skip_group_check=True)
                            m0t = md0p.tile([128, 4, 128], BF16, tag="m0t", name="m0t")
                            nc.scalar.copy(m0t[:, :g, :], ps[:, :g, :])
                            nc.sync.dma_start(
                                out=m_own0_t[t0g:t0g + g].transpose([1, 0, 2]),
                                in_=m0t[:, :g, :])

                def allgather(i):
                    if SKIP_AG:
                        return
                    if SMALL_AG:
                        # timing probe: 2KB payload instead of 1.6MB (numerics wrong)
                        nc.gpsimd.collective_compute(
                            "AllGather", mybir.AluOpType.bypass,
                            ins=[m_own[i][0:8, :]], outs=[m_full[i][0:64, :]],
                            replica_groups=[list(range(P))],
                        )
                        return
                    nc.gpsimd.collective_compute(
                        "AllGather", mybir.AluOpType.bypass,
                        ins=[m_own[i][:]], outs=[m_full[i][:]],
                        replica_groups=[list(range(P))],
                    )

                def graph_layer(layer):
                    """PE one-hot segment sum + relu epilogue -> out_fm."""
                    if SKIP_GRAPH:
                        nc.scalar.activation(out_fm[:], dterm[:], AF.Relu)
                        return
                    with (
                        tc.tile_pool(name=f"gs{rep}_{layer}", bufs=12) as gp,
                        tc.tile_pool(name=f"gi{rep}_{layer}", bufs=1) as gip,
                        tc.tile_pool(name=f"ps{rep}_{layer}", bufs=6, space="PSUM") as psw,
                        tc.tile_pool(name=f"ep{rep}_{layer}", bufs=6) as ep,
                    ):
                        halves = {}
                        for half in (0, 1):
                            Lpad, bounds = meta[(layer, half)]
                            si = gip.tile([128, Lpad // 16], I16, name=f"si{half}",
                                          tag=f"si{half}")
                            nc.sync.dma_start(out=si[:],
                                              in_=idx_d[(layer, half, "s")][:])
                            halves[half] = (Lpad, bounds, si, None, {})
                        nop = len(nops[layer])
                        dr = gip.tile([128, nop, 1], F32, name="dr", tag="dr")
                        nc.sync.dma_start(
                            out=dr[:],
                            in_=idx_d[(layer, "dr")].reshape([128, nop, 1])[:])
                        sall = gip.tile([128, nop, 128], BF16, name="sall",
                                        tag="sall")
                        nc.vector.tensor_tensor(
                            out=sall[:],
                            in0=W["iota128"][:].to_broadcast([128, nop, 128]),
                            in1=dr[:].to_broadcast([128, nop, 128]),
                            op=ALU.is_equal)
                        opctr = [0]

                        src_view = [m_full[layer][0:HALF, :],
                                    m_full[layer][HALF:2 * HALF, :]]

                        def get_chunk(half, c):
                            Lpad, bounds, si, dl, bufs = halves[half]
                            if SKIP_GATHER:
                                if "z" not in bufs:
                                    g = CHUNK // 128
                                    zb = gp.tile([128, g, D], BF16, tag="gbuf", name="gbz")
                                    nc.vector.memset(zb[:], 0.0)
                                    bufs["z"] = zb
                                return bufs["z"]
                            if c not in bufs:
                                g = CHUNK // 128
                                buf = gp.tile([128, g, D], BF16, tag="gbuf",
                                              name=f"gb{half}_{c}")
                                nc.gpsimd.dma_gather(
                                    buf[:], src_view[half],
                                    si[:, c * CHUNK // 16:(c + 1) * CHUNK // 16],
                                    CHUNK, CHUNK, D, queue_num=(2 * c + half) % 4,
                                )
                                bufs[c] = buf
                            return bufs[c]

                        for w0 in range(0, NT, 4):
                            ws = list(range(w0, min(w0 + 4, NT)))
                            pw = psw.tile([128, 512], F32, tag="pw", name="pw")
                            for w in ws:
                                off = (w - w0) * 128
                                ops = []  # (half, tile_idx)
                                for half in (0, 1):
                                    Lpad, bounds, si, dl, bufs = halves[half]
                                    t0 = bounds[w] // 128
                                    t1 = (bounds[w + 1] - 1) // 128
                                    for t in range(t0, t1 + 1):
                                        ops.append((half, t))
                                for i, (half, t) in enumerate(ops):
                                    buf = get_chunk(half, t * 128 // CHUNK)
                                    slot = (t * 128 % CHUNK) // 128
                                    oc = opctr[0]
                                    opctr[0] += 1
                                    nc.tensor.matmul(
                                        pw[:, off:off + 128],
                                        buf[:, slot, :],
                                        sall[:, oc, :],
                                        start=(i == 0), stop=(i == len(ops) - 1),
                                        skip_group_check=True,
                                    )
                            gw = len(ws) * 128
                            blk = slice(w0 * 128, w0 * 128 + gw)
                            sadd = ep.tile([128, 512], F32, tag="sadd", name="sadd")
                            nc.vector.tensor_add(sadd[:, :gw], pw[:, :gw], dterm[:, blk])
                            nc.scalar.activation(out_fm[:, blk], sadd[:, :gw], AF.Relu)

                # ---------------- Layer 0 ----------------
                allgather(0)
                graph_layer(0)
                # m1|d1 from out_fm; overwrite dterm with layer-1 dense term
                with (
                    tc.tile_pool(name=f"md1{rep}", bufs=6) as md1p,
                    tc.tile_pool(name=f"psC{rep}", bufs=4, space="PSUM") as psC,
                ):
                    for lo, w in chunks512():
                        psd = psC.tile([D, 512], F32, tag="d1ps", name="d1ps")
                        nc.tensor.matmul(psd[:, :w], W["w011"][:], out_fm[:, lo:lo + w])
                        nc.scalar.activation(dterm[:, lo:lo + w], psd[:, :w],
                                             AF.Identity, bias=W["ball1"][:])
                    m_own1_t = m_own[1].reshape([NT, 128, D])
                    for t0g in range(0, NT, 4):
                        tg = list(range(t0g, min(t0g + 4, NT)))
                        g = len(tg)
                        ps = psC.tile([128, 4, 128], F32, tag="m1ps", name="m1ps")
                        for j, t in enumerate(tg):
                            lo = t * 128
                            nc.tensor.matmul(ps[:, j, :],
                                             out_fm[:, lo:lo + 128], W["wl1"][:],
                                             skip_group_check=True)
                        m1t = md1p.tile([128, 4, 128], BF16, tag="m1t", name="m1t")
                        nc.scalar.copy(m1t[:, :g, :], ps[:, :g, :])
                        nc.sync.dma_start(
                            out=m_own1_t[t0g:t0g + g].transpose([1, 0, 2]),
                            in_=m1t[:, :g, :])

                # ---------------- Layer 1 ----------------
                allgather(1)
                graph_layer(1)

                # ---------------- Classifier + log_softmax ----------------
                with (
                    tc.tile_pool(name=f"fc{rep}", bufs=6) as fcp,
                    tc.tile_pool(name=f"fcb{rep}", bufs=1) as fcbp,
                    tc.tile_pool(name=f"psD{rep}", bufs=2, space="PSUM") as psD,
                ):
                    tfm = fcbp.tile([128, NPP], F32R, name="tfm")
                    yt_all = fcbp.tile([NCLS, NPP], F32, name="yt_all")
                    for lo, w in chunks512():
                        ps = psD.tile([D, 512], F32, tag="fc1ps", name="fc1ps")
                        nc.tensor.matmul(ps[:, :w], W["fc1s"][:], out_fm[:, lo:lo + w])
                        nc.scalar.activation(tfm[:, lo:lo + w], ps[:, :w], AF.Relu,
                                             bias=W["fc1b"][:])
                    for lo, w in chunks512():
                        ps = psD.tile([NCLS, 512], F32, tag="fc2ps", name="fc2ps")
                        nc.tensor.matmul(ps[:, :w], W["fc2w"][:], tfm[:, lo:lo + w])
                        lg = fcp.tile([NCLS, 512], F32, tag="lg", name="lg")
                        nc.scalar.activation(lg[:, :w], ps[:, :w], AF.Identity,
                                             bias=W["fc2b"][:])
                        ex = fcp.tile([NCLS, 512], F32R, tag="ex", name="ex")
                        nc.scalar.activation(ex[:, :w], lg[:, :w], AF.Exp)
                        ps2 = psD.tile([1, 512], F32, tag="seps", name="seps")
                        nc.tensor.matmul(ps2[:, :w], W["ones40"][:], ex[:, :w])
                        lnt = fcp.tile([1, 512], F32R, tag="lnt", name="lnt")
                        nc.scalar.activation(lnt[:, :w], ps2[:, :w], AF.Ln)
                        ps3 = psD.tile([NCLS, 512], F32, tag="bcps", name="bcps")
                        nc.tensor.matmul(ps3[:, :w], W["ones1x40"][:], lnt[:, :w])
                        nc.vector.tensor_sub(yt_all[:, lo:lo + w], lg[:, :w],
                                             ps3[:, :w])
                    nc.sync.dma_start(out=y_d[:], in_=yt_all[:])
            if REPEAT == 1:
                body(0)
            elif HWLOOP and SKIP_AG:
                # hardware loop (collectives inside For_i fail at runtime)
                with tc.For_i(0, REPEAT):
                    body(0)
            else:
                for rep in range(REPEAT):
                    body(rep)
    nc.compile()
    return nc


def kernel(**inputs):
    global LAST_EXEC_NS, LAST_RESULTS
    h = hashlib.md5()
    for k in sorted(inputs):
        h.update(np.ascontiguousarray(np.asarray(inputs[k])).tobytes())
    key = f"{REPEAT}{SKIP_AG}{SKIP_GATHER}{SKIP_GRAPH}{HWLOOP}{SMALL_AG}" + h.hexdigest()
    if key not in _CACHE:
        in_maps, meta = _prep_inputs(inputs)
        nc = _build({k: (v[0], tuple(v[1])) for k, v in meta.items()})
        _CACHE[key] = (nc, in_maps)
    nc, in_maps = _CACHE[key]
    res = run_bass_kernel_spmd(nc, in_maps, list(range(P)), trace=TRACE, **TRACE_KW)
    LAST_EXEC_NS = res.exec_time_ns
    LAST_RESULTS = res
    outs = res.results
    y = np.concatenate([outs[c]["y"][:, :NPC].T for c in range(P)], axis=0)
    return y.astype(np.float32)

